# revision 5
# baseline (speedup 1.0000x reference)
"""TRN2 Bass kernel for nn_CDF: out[i,j] = order[floor(ndtr(noise[i,j])*N), j].

Architecture (per NeuronCore, 8 cores, column-sharded 32 cols each):
  1. PREP: reblock the core's order slice [N_TRAIN, 32] into DRAM
     blocked[g][s, 64]: 256B blocks = 4 table rows x 16 cols, s = row//4.
  2. For each 256-row batch pair x 16-col group:
     - compute idx = clip(floor(ndtr(noise)*N)) on ACT+DVE,
       s = idx>>2 (fits int16), lo2 = idx&3
     - PE-transpose s into the SWDGE "wrapped" idx layout
       (idxs[16q + n%16, n//16], block order n = c*128 + i per 8-col gather)
     - dma_gather pulls one 256B block per query (1024 idxs/instruction,
       4 SWDGE queues)
     - select-from-4 rows via affine diagonal APs + copy_predicated
  3. Write [128, 32] output tiles back.
"""

import numpy as np

import concourse.bacc as bacc
import concourse.bass as bass
import concourse.mybir as mybir
import concourse.tile as tile
from concourse.bass_utils import run_bass_kernel_spmd
from concourse.masks import make_identity

N_CORES = 8
BATCH = 16384
N_DIM = 256
N_TRAIN = 100000
COLS = N_DIM // N_CORES          # 32 columns per core
P = 128

INV_SQRT2 = 0.7071067811865476

F32 = mybir.dt.float32
I32 = mybir.dt.int32
I16 = mybir.dt.int16
A = mybir.AluOpType


def _prep(nc, pool, order_d, blocked_d, n_train, n_groups):
    """Reblock order slice [n_train, 16*n_groups] -> blocked[g, s, 64]."""
    rows_per_chunk = 4096
    for g in range(n_groups):
        r0 = 0
        while r0 < n_train:
            rows = min(rows_per_chunk, n_train - r0)
            parts = rows // 32
            assert rows % 32 == 0
            stage = pool.tile([P, 512], F32, tag="prep")
            # stage[p, 16*rr + c] = order[r0 + 32p + rr, 16g + c]
            src = order_d.ap()
            src_ap = bass.AP(
                src.tensor,
                src.offset + r0 * (16 * n_groups) + 16 * g,
                [[32 * 16 * n_groups, parts], [16 * n_groups, 32], [1, 16]],
            )
            nc.sync.dma_start(
                stage[:parts, :].rearrange("p (rr c) -> p rr c", c=16), src_ap)
            # blocked[g, r0//4 + 8p + rr2, 16r + c] = stage[p, 64rr2 + 16r + c]
            dst = blocked_d.ap()
            dst_ap = bass.AP(
                dst.tensor,
                dst.offset + (g * (n_train // 4) + r0 // 4) * 64,
                [[8 * 64, parts], [64, 8], [1, 64]],
            )
            nc.sync.dma_start(
                dst_ap,
                stage[:parts, :].rearrange("p (rr2 x) -> p rr2 x", x=64))
            r0 += rows


def build_nc(batch=BATCH, n_train=N_TRAIN, cols=COLS, nq=4,
             act_fn=None):
    assert n_train % 4 == 0
    n_groups = cols // 16
    ns = n_train // 4
    assert ns - 1 <= 32767
    n_pairs = batch // 256

    nc = bacc.Bacc("TRN2", target_bir_lowering=False, debug=False,
                   num_swdge_queues=nq, dynamic_dma_scratch_size=2 ** 16)
    noise_d = nc.dram_tensor("noise", [batch, cols], F32, kind="ExternalInput")
    order_d = nc.dram_tensor("order", [n_train, cols], F32,
                             kind="ExternalInput")
    out_d = nc.dram_tensor("out", [batch, cols], F32, kind="ExternalOutput")
    blocked_d = nc.dram_tensor("blocked", [n_groups * ns, 64], F32,
                               kind="Internal")

    gq = [0]

    with tile.TileContext(nc) as tc:
        with tc.tile_pool(name="const", bufs=1) as cpool, \
             tc.tile_pool(name="prep", bufs=4) as ppool, \
             tc.tile_pool(name="work", bufs=3) as wpool, \
             tc.tile_pool(name="idxp", bufs=3) as ipool, \
             tc.tile_pool(name="gath", bufs=8) as gpool, \
             tc.tile_pool(name="psum", bufs=2, space="PSUM") as pspool:

            ident = cpool.tile([P, P], F32)
            make_identity(nc, ident[:])

            _prep(nc, ppool, order_d, blocked_d, n_train, n_groups)

            def do_pair(pair, g16):
                i0 = pair * 256
                # ---- load noise [128, 32]: [p, 16*sub + c] ----
                x = wpool.tile([P, 32], F32, tag="x")
                nap = noise_d.ap()
                src_ap = bass.AP(
                    nap.tensor, nap.offset + i0 * cols + 16 * g16,
                    [[cols, P], [P * cols, 2], [1, 16]],
                )
                nc.sync.dma_start(
                    x[:].rearrange("p (s c) -> p s c", c=16), src_ap)

                # ---- index chain ----
                e = wpool.tile([P, 32], F32, tag="e")
                nc.scalar.activation(e[:], x[:],
                                     act_fn or mybir.ActivationFunctionType.Erf,
                                     scale=INV_SQRT2)
                tf = wpool.tile([P, 32], F32, tag="tf")
                nc.vector.tensor_scalar(tf[:], e[:], 0.5 * n_train,
                                        0.5 * n_train - 0.5, A.mult, A.add)
                ti = wpool.tile([P, 32], I32, tag="ti")
                nc.vector.tensor_copy(ti[:], tf[:])
                nc.vector.tensor_scalar(ti[:], ti[:], n_train - 1, 0,
                                        A.min, A.max)
                lo2 = wpool.tile([P, 32], I32, tag="lo2")
                nc.vector.tensor_scalar(lo2[:], ti[:], 3, None, A.bitwise_and)
                s32 = wpool.tile([P, 32], I32, tag="s32")
                nc.vector.tensor_scalar(s32[:], ti[:], 2, None,
                                        A.arith_shift_right)
                sf = wpool.tile([P, 32], F32, tag="sf")
                nc.vector.tensor_copy(sf[:], s32[:])

                # ---- T1: PE transpose sf [128, 32] -> PSUM [32, 128] ----
                xt_ps = pspool.tile([32, P], F32, tag="t1")
                nc.tensor.transpose(xt_ps[:], sf[:], ident[:])
                X = wpool.tile([32, P], F32, tag="X")
                nc.vector.tensor_copy(X[:], xt_ps[:])

                # ---- T2 (per w): [32, 16] -> [16, 32] -> idxs ----
                idxs = ipool.tile([P, 256], I16, tag="idxs")
                for w in range(8):
                    t2_ps = pspool.tile([16, 32], F32, tag="t2")
                    nc.tensor.transpose(t2_ps[:], X[:, 16 * w:16 * w + 16],
                                        ident[:32, :32])
                    # idxs[a, 128*sub + 8c + w] = t2[a, 16*sub + c]
                    iap = idxs[:]
                    dst = bass.AP(iap.tensor, iap.offset + w,
                                  [[256, 16], [128, 2], [8, 16]])
                    nc.vector.tensor_copy(
                        dst, t2_ps[:].rearrange("a (s c) -> a s c", c=16))

                # ---- replicate idxs to all 8 q-groups ----
                for sz in (16, 32, 64):
                    nc.sync.dma_start(idxs[sz:2 * sz, :], idxs[0:sz, :])

                # ---- masks (shared) ----
                masks = []
                for r in range(1, 4):
                    m = wpool.tile([P, 32], I32, tag=f"m{r}")
                    nc.vector.tensor_scalar(m[:], lo2[:], r, None, A.is_equal)
                    masks.append(m)

                # ---- gathers + select ----
                acc = wpool.tile([P, 32], F32, tag="acc")
                for sub in range(2):
                    for h in range(2):
                        gidx = 2 * sub + h
                        g = gpool.tile([P, 512], F32, tag="g")
                        nc.gpsimd.dma_gather(
                            out_ap=g[:].rearrange("p (n x) -> p n x", x=64),
                            in_ap=blocked_d.ap()[g16 * ns:(g16 + 1) * ns, :],
                            idxs_ap=idxs[:, 64 * gidx:64 * gidx + 64],
                            num_idxs=1024,
                            num_idxs_reg=1024,
                            elem_size=64,
                            queue_num=gq[0] % nq,
                        )
                        gq[0] += 1
                        gv = g[:]
                        accoff = 16 * sub + 8 * h

                        def cand(r):
                            return bass.AP(gv.tensor,
                                           gv.offset + 16 * r + 8 * h,
                                           [list(gv.ap[0]), [65, 8]])

                        acc_sl = acc[:, accoff:accoff + 8]
                        nc.vector.tensor_copy(acc_sl, cand(0))
                        for r in range(1, 4):
                            nc.vector.copy_predicated(
                                acc_sl, masks[r - 1][:, accoff:accoff + 8],
                                cand(r))

                # ---- write out [128, 32] -> out rows ----
                oap = out_d.ap()
                dst_ap = bass.AP(
                    oap.tensor, oap.offset + i0 * cols + 16 * g16,
                    [[cols, P], [P * cols, 2], [1, 16]],
                )
                nc.sync.dma_start(
                    dst_ap, acc[:].rearrange("p (s c) -> p s c", c=16))

            for pair in range(n_pairs):
                for g16 in range(n_groups):
                    do_pair(pair, g16)

    nc.compile()
    return nc


_nc_cache = {}


def _get_nc():
    if "nc" not in _nc_cache:
        _nc_cache["nc"] = build_nc()
    return _nc_cache["nc"]


def kernel(noise: np.ndarray, order: np.ndarray) -> np.ndarray:
    noise = np.ascontiguousarray(np.asarray(noise, dtype=np.float32))
    order = np.ascontiguousarray(np.asarray(order, dtype=np.float32))
    assert noise.shape == (BATCH, N_DIM)
    assert order.shape == (N_TRAIN, N_DIM)
    nc = _get_nc()
    in_maps = [
        {
            "noise": np.ascontiguousarray(noise[:, c * COLS:(c + 1) * COLS]),
            "order": np.ascontiguousarray(order[:, c * COLS:(c + 1) * COLS]),
        }
        for c in range(N_CORES)
    ]
    res = run_bass_kernel_spmd(nc, in_maps, core_ids=list(range(N_CORES)))
    return np.concatenate([r["out"] for r in res.results], axis=1)
